# revision 13
# baseline (speedup 1.0000x reference)
"""Trainium2 Bass kernel for nn_CLLayer_47064251630125 (contrastive loss).

Reference computation (per row i of N=8192):
    h1 = ELU(z1 @ W1.T + b1) @ W2.T + b2 ; h2 likewise
    na = normalize(h1), nb = normalize(h2)   (L2 row norm)
    l1 = -log( exp(2 na_i.nb_i) / (sum_j exp(2 na_i.na_j) + sum_j exp(2 na_i.nb_j) - e^2) )
    l2 = same with roles swapped (uses column sums of the cross matrix)
    out = (l1 + l2)/2

Sharding (all-gather design, per the spec hint): each core receives ONLY its
own 1024-row block of z1/z2 (transposed, [256 x 1024] bf16), projects and
normalizes just that block, then AllGathers the fp8-quantized normalized
projections so every core holds the full [256 x 8192] na/nb.  This cuts
host->device staging 7.3x (75.5MB -> 10.3MB per dispatch) and projection
compute 8x vs. replicating full z everywhere.

Schedule: z2 is projected FIRST and its normalized block is gathered in two
512-column chunk collectives (nb feeds BOTH the R2 and B streams; chunked
gather lets the similarity pipeline start ~40us earlier).  z1/na follows as
one whole-block gather - na's only remote use is stream R1, which runs last.
Gathered column order is chunk-major (chunk, core, 512); row sums don't care,
and the one consumer that does (B's column sums) un-permutes statically when
staging the ReduceScatter input.

Similarity streams over [block x full] use fp8e4 DoubleRow matmuls (two
128-row k-tiles contracted per instruction at 0.5 cyc/row = 4x the bf16
rate) with fused exp+row-sum on the scalar engine (activation accum_out):
  R2 = (nb, nb)  -> denom2 refl term       (2048-wide groups)
  B  = (na, nb)  -> denom1 cross term; its fp8 exp tiles are also
                    column-summed (DoubleRow ones matmuls over isub pairs)
                    and ReduceScatter'ed for denom2's cross term
  R1 = (na, na)  -> denom1 refl term       (2048-wide groups)
The positive-pair term uses the local fp8 blocks: log(exp(2 d)) = 2 d.
fp8 quantization perturbs each similarity dot by ~3e-3 abs -> ~1e-3 rel
on the loss, well inside the 2e-2 gate.
"""

import sys

sys.path.insert(0, "/opt/trn_rl_repo")

import numpy as np
import ml_dtypes

import concourse.bass as bass
import concourse.mybir as mybir
import concourse.tile as tile
from concourse import bacc
import concourse.hw_specs as _hw_specs
import concourse.bass_interp as _interp
import concourse.bacc as _bacc_mod

# This kernel only uses exp and ln on the ACT engine, both of which live in
# the natural_log_exp_and_others table set.  The greedy table-load placement
# pass otherwise flip-flops between per-function sets (1.28us per swap, and
# the swaps land on the critical path).  Empty every other set so the
# fixpoint pins the one combined set; list positions (= act_func_set_id)
# are preserved.
_orig_gat = _hw_specs.get_activation_tables


def _gat_nle_only(arch):
    return {name: (funcs if name == "natural_log_exp_and_others" else set())
            for name, funcs in _orig_gat(arch).items()}


for _m in (_hw_specs, _interp, _bacc_mod):
    _m.get_activation_tables = _gat_nle_only

BF16 = mybir.dt.bfloat16
FP8 = mybir.dt.float8e4
F32 = mybir.dt.float32
AF = mybir.ActivationFunctionType
ALU = mybir.AluOpType
DR = mybir.MatmulPerfMode.DoubleRow

P = 128
D = 256
KT = D // P          # 2 k-tiles
N_FULL = 8192
N_CORES = 8
CH = 512             # free-dim chunk per matmul / projection chunk
TAU = 0.5
SIM_SCALE = 1.0 / TAU      # 2.0
E2 = float(np.exp(SIM_SCALE))  # exp(2 * ||na||^2) ~ e^2, diag of refl


def build_bass(n_full=N_FULL, blk=None, n_cores=N_CORES):
    """Trace the Tile kernel.  Returns the compiled Bacc object (SPMD)."""
    if blk is None:
        blk = n_full // n_cores
    CB = blk // CH               # projection chunks per block
    ISUB = blk // P              # i-subtiles per core block
    NPAIR = ISUB // 2            # isub pairs for cs DoubleRow matmuls
    GQ = CH // 2                 # nb gather granularity (columns per core)
    NQ = blk // GQ               # nb quarter-collectives
    GWQ = n_cores * GQ           # gathered columns per nb quarter
    BL2 = blk // 2               # na gather granularity (columns per core)
    GWH = n_cores * BL2          # gathered columns per na half
    G2 = min(2048, GWQ)          # refl-stream j-group width
    G1 = min(1024, GWQ)          # B-stream j-group width
    NJ = n_full // 1024          # rs slot granularity (1024 columns)

    nc = bacc.Bacc("TRN2", target_bir_lowering=False, debug=False,
                   num_devices=n_cores)

    z1b = nc.dram_tensor("z1b", [D, blk], BF16, kind="ExternalInput")
    z2b = nc.dram_tensor("z2b", [D, blk], BF16, kind="ExternalInput")
    w1t = nc.dram_tensor("w1t", [D, D], BF16, kind="ExternalInput")
    w2t = nc.dram_tensor("w2t", [D, D], BF16, kind="ExternalInput")
    b1d = nc.dram_tensor("b1", [D], F32, kind="ExternalInput")
    b2d = nc.dram_tensor("b2", [D], F32, kind="ExternalInput")
    out = nc.dram_tensor("out", [P, ISUB], F32, kind="ExternalOutput")

    with tile.TileContext(nc) as tc:
        with (
            tc.tile_pool(name="const", bufs=1) as cpool,
            tc.tile_pool(name="persist", bufs=1) as ppool,
            tc.tile_pool(name="io", bufs=4) as iopool,
            tc.tile_pool(name="scratch", bufs=4) as spool,
            tc.tile_pool(name="dram", bufs=1, space="DRAM") as dpool,
        ):
            # ---- constants ----
            w1_sb = cpool.tile([P, KT, D], BF16)
            nc.sync.dma_start(w1_sb, w1t.rearrange("(k p) c -> p k c", p=P))
            w2_sb = cpool.tile([P, KT, D], BF16)
            nc.sync.dma_start(w2_sb, w2t.rearrange("(k p) c -> p k c", p=P))
            b1f = cpool.tile([P, KT], F32)
            nc.sync.dma_start(b1f, b1d.rearrange("(m p) -> p m", p=P))
            b2f = cpool.tile([P, KT], F32)
            nc.sync.dma_start(b2f, b2d.rearrange("(m p) -> p m", p=P))
            # derived bias forms for the relu path: relu(x+b) = max(x,-b)+b
            nb1 = cpool.tile([P, KT], F32)
            nc.vector.tensor_scalar_mul(nb1, b1f, -1.0)
            b1p1 = cpool.tile([P, KT], F32)
            nc.vector.tensor_scalar_add(b1p1, b1f, 1.0)
            ones_col = cpool.tile([P, 1], BF16)
            nc.vector.memset(ones_col, 1.0)
            ones_row = cpool.tile([1, P], BF16)
            nc.vector.memset(ones_row, 1.0)
            # DoubleRow needs >=16 output partitions: 16 duplicate ones
            # columns; row 0 of the column-sum result is read back.
            ones2_f8 = cpool.tile([P, KT, 16], FP8)
            nc.vector.memset(ones2_f8, 1.0)
            # pin the natural_log_exp ACT table set (has exp AND ln) under
            # the input DMAs so no swaps occur later
            warm = cpool.tile([P, 1], BF16)
            nc.scalar.activation(warm, ones_col, AF.Ln)
            nc.scalar.activation(warm, ones_col, AF.Exp)

            rs = ppool.tile([P, 4, ISUB * NJ], F32)
            nc.vector.memset(rs, 0.0)

            # DRAM staging for the collectives (all fp8): nb per quarter,
            # na per half
            ag_space = "Shared" if n_cores > 4 else "Local"
            agi_nb = [dpool.tile([KT * P * GQ], FP8, name=f"aginb{q}")
                      for q in range(NQ)]
            ago_nb = [dpool.tile([n_cores * KT * P * GQ], FP8,
                                 name=f"agonb{q}", addr_space=ag_space)
                      for q in range(NQ)]
            agi_na = [dpool.tile([KT * P * BL2], FP8, name=f"agina{j}")
                      for j in range(2)]
            ago_na = [dpool.tile([n_cores * KT * P * BL2], FP8,
                                 name=f"agona{j}", addr_space=ag_space)
                      for j in range(2)]
            cc_in = dpool.tile([n_full], F32, name="cc_in")
            cc_out = dpool.tile([blk], F32, name="cc_out")
            cs_all = ppool.tile([1, n_full], F32, name="cs_all")

            # ============ project own block (z2 FIRST, then z1) ============
            # nf8 [P, KT, blk] fp8: feature d = k*128+p, col = row idx
            fin = ppool.tile([P, 10, ISUB], F32)
            blk8 = {}
            with tc.tile_pool(name="psA", bufs=1, space="PSUM") as psA:
                for idx, zt in ((1, z2b), (0, z1b)):
                    zt_ap = zt.rearrange("(k p) w -> p k w", p=P)
                    nf8 = ppool.tile([P, KT, blk], FP8, name=f"nf8{idx}",
                                     tag=f"nf8{idx}")
                    blk8[idx] = nf8
                    for c in range(CB):
                        cs = slice(c * CH, (c + 1) * CH)
                        zch = iopool.tile([P, KT, CH], BF16, tag="zch")
                        nc.sync.dma_start(zch, zt_ap[:, :, cs])
                        # L1: pa[m] = W1 @ z.T  (biases in the epilogues)
                        pa = psA.tile([P, KT, CH], F32, name="pa", tag="pa",
                                      bufs=2)
                        for m in range(KT):
                            ms = slice(m * P, (m + 1) * P)
                            for k in range(KT):
                                nc.tensor.matmul(pa[:, m], w1_sb[:, k, ms],
                                                 zch[:, k], start=(k == 0),
                                                 stop=(k == KT - 1))
                        # ELU' = elu+1 = min(exp(x+b1), relu(x+b1)+1)
                        # (the -1 is folded into b2 on the host)
                        e_t = spool.tile([P, KT, CH], BF16, tag="e")
                        r_t = spool.tile([P, KT, CH], BF16, tag="r")
                        aT = spool.tile([P, KT, CH], BF16, tag="aT")
                        for m in range(KT):
                            nc.scalar.activation(e_t[:, m], pa[:, m], AF.Exp,
                                                 bias=b1f[:, m:m + 1])
                            nc.vector.tensor_scalar(r_t[:, m], pa[:, m],
                                                    nb1[:, m:m + 1],
                                                    b1p1[:, m:m + 1],
                                                    ALU.max, ALU.add)
                            nc.vector.tensor_tensor(aT[:, m], e_t[:, m],
                                                    r_t[:, m], ALU.min)
                        # L2: ph[m2] = W2 @ a
                        ph = psA.tile([P, KT, CH], F32, name="ph", tag="ph",
                                      bufs=1)
                        for m2 in range(KT):
                            ms = slice(m2 * P, (m2 + 1) * P)
                            for m in range(KT):
                                nc.tensor.matmul(ph[:, m2], w2_sb[:, m, ms],
                                                 aT[:, m], start=(m == 0),
                                                 stop=(m == KT - 1))
                        # h = ph + b2 -> sbuf bf16; sq = h*h; norms on PE
                        hc = spool.tile([P, KT, CH], BF16, tag="hc")
                        sq = spool.tile([P, KT, CH], BF16, tag="sq")
                        for m2 in range(KT):
                            nc.vector.tensor_scalar(hc[:, m2], ph[:, m2],
                                                    b2f[:, m2:m2 + 1],
                                                    None, ALU.add)
                            nc.vector.tensor_tensor(sq[:, m2], hc[:, m2],
                                                    hc[:, m2], ALU.mult)
                        ns = psA.tile([1, CH], F32, name="ns", tag="small",
                                      bufs=2)
                        for m2 in range(KT):
                            nc.tensor.matmul(ns, ones_col, sq[:, m2],
                                             start=(m2 == 0),
                                             stop=(m2 == KT - 1))
                        # rn = ns^-1/2 = exp(-0.5 ln ns)  (same ACT table
                        # set as exp: no swap); broadcast across partitions
                        # with a rank-1 PE matmul, then normalize+quantize
                        lns = spool.tile([1, CH], F32, tag="lns")
                        nc.scalar.activation(lns, ns, AF.Ln)
                        rn_sb = spool.tile([1, CH], BF16, tag="rnsb")
                        nc.scalar.activation(rn_sb, lns, AF.Exp, scale=-0.5)
                        rn_ps = psA.tile([P, CH], F32, name="rn_ps",
                                         tag="small", bufs=2)
                        nc.tensor.matmul(rn_ps, ones_row, rn_sb,
                                         start=True, stop=True)
                        for k in range(KT):
                            nc.vector.tensor_tensor(nf8[:, k, cs], hc[:, k],
                                                    rn_ps, ALU.mult)
                        # stage + gather every piece this chunk completes
                        # (z2 -> GQ-quarters, z1 -> BL2-halves)
                        gran = GQ if idx == 1 else BL2
                        agi = agi_nb if idx == 1 else agi_na
                        ago = ago_nb if idx == 1 else ago_na
                        done = ((c + 1) * CH) // gran
                        for g in range((c * CH) // gran, done):
                            gs = slice(g * gran, (g + 1) * gran)
                            nc.sync.dma_start(
                                agi[g].rearrange("(k p b) -> p k b",
                                                 p=P, k=KT),
                                nf8[:, :, gs])
                            nc.gpsimd.collective_compute(
                                "AllGather", ALU.bypass,
                                replica_groups=[list(range(n_cores))],
                                ins=[agi[g][:]], outs=[ago[g][:]])
                # positive pairs from the local fp8 blocks (both ready now)
                pd = spool.tile([P, KT, blk], BF16, tag="pd")
                for k in range(KT):
                    nc.vector.tensor_tensor(pd[:, k], blk8[0][:, k],
                                            blk8[1][:, k], ALU.mult)
                pos_ps = psA.tile([P, ISUB], F32, name="pos_ps", bufs=2,
                                  tag="small")
                for s in range(ISUB):
                    ss = slice(s * P, (s + 1) * P)
                    for k in range(KT):
                        nc.tensor.matmul(pos_ps[:, s:s + 1], pd[:, k, ss],
                                         ones_col, start=(k == 0),
                                         stop=(k == KT - 1))
                nc.vector.tensor_copy(fin[:, 9], pos_ps)

            # gathered full tensors; nb quarter q covers gathered columns
            # [q*GWQ, (q+1)*GWQ) in (core, GQ) order
            nbT = ppool.tile([P, KT, n_full], FP8, name="nbT")
            for q in range(NQ):
                agv = ago_nb[q].rearrange("(c k p b) -> p k c b",
                                          c=n_cores, k=KT, p=P)
                for k in range(KT):
                    nc.sync.dma_start(
                        nbT[:, k, q * GWQ:(q + 1) * GWQ].rearrange(
                            "p (c b) -> p c b", c=n_cores),
                        agv[:, k])

            def emit_group(st, lhs8, rhs8, psS, nbufs, isub, jg, grp,
                           egp=None, cs_tiles=None):
                """One [128 x grp] similarity tile: fp8 DoubleRow matmuls +
                fused exp/row-sum.  With cs_tiles, exp goes to the paired
                fp8 tile egp[:, isub%2]; after odd isubs a DoubleRow ones
                matmul accumulates column sums of both tiles."""
                gb = grp // CH
                isl = slice(isub * P, (isub + 1) * P)
                pg = psS.tile([P, gb, CH], F32, tag="sgrp", bufs=nbufs,
                              name="pg")
                for js in range(gb):
                    jss = slice(jg * grp + js * CH, jg * grp + (js + 1) * CH)
                    nc.tensor.matmul(pg[:, js], lhs8[:, :, isl],
                                     rhs8[:, :, jss], start=True, stop=True,
                                     perf_mode=DR)
                col = isub * NJ + (jg * grp) // 1024
                if cs_tiles is None:
                    eg = spool.tile([P, gb, CH], FP8, tag=f"eg{gb}")
                    nc.scalar.activation(eg, pg, AF.Exp, scale=SIM_SCALE,
                                         accum_out=rs[:, st, col:col + 1])
                else:
                    par = isub % 2
                    nc.scalar.activation(egp[:, par], pg, AF.Exp,
                                         scale=SIM_SCALE,
                                         accum_out=rs[:, st, col:col + 1])
                    if par == 1:
                        pr = isub // 2
                        for js in range(gb):
                            nc.tensor.matmul(cs_tiles[js], ones2_f8,
                                             egp[:, :, js],
                                             start=(pr == 0),
                                             stop=(pr == NPAIR - 1),
                                             perf_mode=DR)
                        if pr == NPAIR - 1:
                            for seg in range(grp // GQ):
                                # gathered col -> owner-natural position
                                g0 = jg * grp + seg * GQ
                                cpos = ((g0 % GWQ) // GQ) * blk \
                                    + (g0 // GWQ) * GQ
                                so = (seg * GQ) % CH
                                nc.vector.tensor_copy(
                                    cs_all[0:1, cpos:cpos + GQ],
                                    cs_tiles[(seg * GQ) // CH][0:1,
                                                               so:so + GQ])

            # ---- streams R2 + B interleaved, paced by the nb quarters ----
            naT = ppool.tile([P, KT, n_full], FP8, name="naT")
            for q in range(NQ):
                with tc.tile_pool(name="psR2", bufs=1, space="PSUM") as psR2:
                    for jh in range(GWQ // G2):
                        jg = q * (GWQ // G2) + jh
                        for isub in range(ISUB):
                            emit_group(3, blk8[1], nbT, psR2, 2, isub, jg, G2)
                with tc.tile_pool(name="psB", bufs=1, space="PSUM") as psB:
                    cs_tiles = [psB.tile([16, CH], F32, name=f"cst{js}",
                                         tag=f"cst{js}", bufs=1)
                                for js in range(G1 // CH)]
                    for jh in range(GWQ // G1):
                        jg = q * (GWQ // G1) + jh
                        egp = spool.tile([P, 2, G1 // CH, CH], FP8,
                                         tag="egp", bufs=2)
                        for isub in range(ISUB):
                            emit_group(1, blk8[0], nbT, psB, 3, isub, jg, G1,
                                       egp=egp, cs_tiles=cs_tiles)

            # na halves land while the later quarters stream
            for j in range(2):
                agv = ago_na[j].rearrange("(c k p b) -> p k c b",
                                          c=n_cores, k=KT, p=P)
                for k in range(KT):
                    nc.sync.dma_start(
                        naT[:, k, j * GWH:(j + 1) * GWH].rearrange(
                            "p (c b) -> p c b", c=n_cores),
                        agv[:, k])

            nc.sync.dma_start(cc_in[None, :], cs_all)
            nc.gpsimd.collective_compute(
                "ReduceScatter", ALU.add,
                replica_groups=[list(range(n_cores))],
                ins=[cc_in[:]], outs=[cc_out[:]])

            # d2-side prep overlaps stream R1
            ccv = ppool.tile([P, ISUB], F32)
            nc.sync.dma_start(ccv, cc_out.rearrange("(s p) -> p s", p=P))
            rs4 = rs.rearrange("p s (i j) -> p s i j", j=NJ)

            with tc.tile_pool(name="psR1", bufs=1, space="PSUM") as psR1:
                for jg in range(n_full // G2):
                    for isub in range(ISUB):
                        emit_group(0, blk8[0], naT, psR1, 2, isub, jg, G2)

            # ---- final: l = 0.5*ln(d1*d2) - 2*pos ----
            for st in (0, 1, 3):
                nc.vector.tensor_reduce(out=fin[:, st, :, None],
                                        in_=rs4[:, st], op=ALU.add,
                                        axis=mybir.AxisListType.X)
            nc.vector.scalar_tensor_tensor(fin[:, 4], fin[:, 0], -E2,
                                           fin[:, 1], ALU.add, ALU.add)
            nc.vector.scalar_tensor_tensor(fin[:, 5], fin[:, 3], -E2,
                                           ccv, ALU.add, ALU.add)
            nc.vector.tensor_tensor(fin[:, 6], fin[:, 4], fin[:, 5],
                                    ALU.mult)
            nc.scalar.activation(fin[:, 8], fin[:, 6], AF.Ln)
            lres = ppool.tile([P, ISUB], F32)
            nc.vector.scalar_tensor_tensor(lres, fin[:, 9],
                                           -2.0 * SIM_SCALE, fin[:, 8],
                                           ALU.mult, ALU.add)
            nc.vector.tensor_scalar_mul(lres, lres, 0.5)
            nc.sync.dma_start(out[:, :], lres)

    nc.compile()
    return nc


def prep_inputs(z1, z2, W1, b1, W2, b2, n_full=N_FULL, n_cores=N_CORES):
    """Host-side prep -> list of per-core input maps (numpy)."""
    blk = n_full // n_cores
    bf = ml_dtypes.bfloat16
    z1t = np.ascontiguousarray(z1.T).astype(bf)
    z2t = np.ascontiguousarray(z2.T).astype(bf)
    w1t = np.ascontiguousarray(W1.T).astype(bf)
    w2t = np.ascontiguousarray(W2.T).astype(bf)
    # ELU' = elu + 1 is used as the L1 activation; fold the "-1" into b2:
    # h = W2 @ (elu'(x) - 1) + b2 = W2 @ elu'(x) + (b2 - W2.sum(1))
    b2_eff = (b2 - W2.sum(axis=1)).astype(np.float32)
    b1c = b1.astype(np.float32)
    in_maps = []
    for c in range(n_cores):
        bs = slice(c * blk, (c + 1) * blk)
        in_maps.append({
            "z1b": np.ascontiguousarray(z1t[:, bs]),
            "z2b": np.ascontiguousarray(z2t[:, bs]),
            "w1t": w1t, "w2t": w2t, "b1": b1c, "b2": b2_eff,
        })
    return in_maps


_NC_CACHE = {}


def _get_nc(n_full=N_FULL, n_cores=N_CORES):
    key = (n_full, n_cores)
    if key not in _NC_CACHE:
        _NC_CACHE[key] = build_bass(n_full=n_full, n_cores=n_cores)
    return _NC_CACHE[key]


def kernel(z1, z2, W1, b1, W2, b2):
    from concourse.bass_utils import run_bass_kernel_spmd

    n_full = z1.shape[0]
    n_cores = N_CORES
    in_maps = prep_inputs(z1, z2, W1, b1, W2, b2, n_full, n_cores)
    nc = _get_nc(n_full, n_cores)
    res = run_bass_kernel_spmd(nc, in_maps, core_ids=list(range(n_cores)))
    parts = [np.asarray(res.results[c]["out"]).T.reshape(-1)
             for c in range(n_cores)]
    return np.concatenate(parts).astype(np.float32)


if __name__ == "__main__":
    nc = build_bass()
    print("traced ok")
